# revision 1
# baseline (speedup 1.0000x reference)
import numpy as np

EPS = 1e-5
NC = 8
H = W = 3
N = 9
OC = 32
CP = 7
HN = 256
P = 128  # partitions


def _fold_consts(inp):
    """Host-side folding of all weights into matmul-ready constants."""
    f = lambda k: np.asarray(inp[k], np.float32)
    # image branch
    s1 = f('g1') / np.sqrt(f('v1') + EPS)
    A1 = f('w1')[:, 0] * f('wv')[0, 0] * s1 if 'wv' in inp else None
    return None


def _build(inputs):
    import concourse.bass as bass
    import concourse.bacc as bacc
    import concourse.tile as tile
    from concourse import mybir
    from concourse.bass_utils import run_bass_kernel_spmd

    dt = mybir.dt
    AF = mybir.ActivationFunctionType
    ALU = mybir.AluOpType

    x = np.asarray(inputs['x'], np.float32)
    xp = np.asarray(inputs['x_param'], np.float32)
    B = x.shape[0]
    Bc = B // NC            # rows per core
    G = Bc // P             # 128 g-groups per partition

    g = lambda k: np.asarray(inputs[k], np.float32)

    # ---------------- host-side constant folding ----------------
    # image branch (CIN=1, IC=1)
    wq, wk, wv = g('wq'), g('wk'), g('wv')
    c0 = float(wq[0, 0] * wk[0, 0])           # energy scale for image branch
    s1 = g('g1') / np.sqrt(g('v1') + EPS)
    A1 = g('w1')[:, 0] * wv[0, 0] * s1        # [32]
    C1 = (g('b1') - g('m1')) * s1 + g('be1')
    s2 = g('g2') / np.sqrt(g('v2') + EPS)
    W2i = g('w2') * s2[:, None]               # [32,32] row-scaled
    C2i = (g('b2') - g('m2')) * s2 + g('be2')
    # param branch
    wqp, wkp, wvp = g('wqp'), g('wkp'), g('wvp')
    s1p = g('g1p') / np.sqrt(g('v1p') + EPS)
    W1v = (g('w1p') * s1p[:, None]) @ wvp     # [32,7]
    C1p = (g('b1p') - g('m1p')) * s1p + g('be1p')
    s2p = g('g2p') / np.sqrt(g('v2p') + EPS)
    W2p = g('w2p') * s2p[:, None]
    C2p = (g('b2p') - g('m2p')) * s2p + g('be2p')
    fw1, fb1, fw2, fb2 = g('fw1'), g('fb1'), g('fw2'), g('fb2')

    # y1cat feature order: j<32 param branch, j>=32 image branch
    W1all = np.zeros((8, 64), np.float32)
    W1all[0:7, 0:32] = W1v.T                  # lhsT[k=c, m=j] = W1v[j, c]
    W1all[7, 32:64] = A1
    b1all = np.concatenate([C1p, C1]).astype(np.float32)          # [64]
    W1pair = np.zeros((16, 128), np.float32)
    W1pair[0:8, 0:64] = W1all
    W1pair[8:16, 64:128] = W1all
    b1pair = np.concatenate([b1all, b1all]).astype(np.float32)    # [128]

    W2s = np.zeros((64, 64), np.float32)      # lhsT[k=y1feat, m=y2feat]
    W2s[0:32, 0:32] = W2p.T
    W2s[32:64, 32:64] = W2i.T
    b2all = np.concatenate([C2p, C2i]).astype(np.float32)
    W2pair = np.zeros((128, 128), np.float32)
    W2pair[0:64, 0:64] = W2s
    W2pair[64:128, 64:128] = W2s
    b2pair = np.concatenate([b2all, b2all]).astype(np.float32)

    # fc1 weight rearranged per position i: rows = (i_local, j), cols = hn
    def catidx(j, i):
        if j < 32:
            return 288 + j * 9 + i            # param block of cat
        return (j - 32) * 9 + i               # image block of cat

    M1 = []                                   # 4 pair tiles [128,256] + single [64,256]
    for t in range(4):
        m = np.zeros((128, 256), np.float32)
        for ii in range(2):
            i = 2 * t + ii
            for j in range(64):
                m[ii * 64 + j, :] = fw1[:, catidx(j, i)]
        M1.append(m)
    m = np.zeros((64, 256), np.float32)
    for j in range(64):
        m[j, :] = fw1[:, catidx(j, 8)]
    M1.append(m)

    fw2T = fw2.T.astype(np.float32)           # [256, 2]

    # ---- pack weight constants into one [128, Fw] tensor ----
    cols = {}
    off = 0
    def put(name, arr, row0=0):
        nonlocal off
        a = np.zeros((128, arr.shape[1]), np.float32)
        a[row0:row0 + arr.shape[0]] = arr
        cols[name] = (off, arr.shape[1], row0 + arr.shape[0], row0)
        off += arr.shape[1]
        return a
    blocks = []
    blocks.append(put('id', np.eye(128, dtype=np.float32)))
    blocks.append(put('w2pair', W2pair))
    for t in range(4):
        q = np.zeros((64, 128), np.float32)
        q[16 * t:16 * t + 16, :] = W1pair
        blocks.append(put(f'w1quad{t}', q))
    blocks.append(put('w1s', W1all, row0=64))
    blocks.append(put('w2s', W2s))
    for t in range(5):
        blocks.append(put(f'm1_{t}a', M1[t][:, 0:128]))
        blocks.append(put(f'm1_{t}b', M1[t][:, 128:256]))
    blocks.append(put('fw2a', fw2T[0:128]))
    blocks.append(put('fw2b', fw2T[128:256]))
    cw_np = np.concatenate(blocks, axis=1)
    import ml_dtypes
    cw_np_bf = cw_np.astype(ml_dtypes.bfloat16)
    Fw = cw_np.shape[1]

    cb_np = np.zeros((128, 8), np.float32)
    cb_np[:, 0] = b1pair
    cb_np[:, 1] = b2pair
    cb_np[:, 2] = fb1[0:128]
    cb_np[:, 3] = fb1[128:256]
    cb_np[0:2, 4] = fb2 * 0.5
    cb_np[0:64, 5] = b1all
    cb_np[0:64, 6] = b2all

    # ---------------- build the bass program ----------------
    nc = bacc.Bacc("TRN2", target_bir_lowering=False, debug=False)
    f32, f32r, bf16 = dt.float32, dt.float32r, dt.bfloat16

    x_d = nc.dram_tensor("xin", [Bc * 9], f32, kind="ExternalInput").ap()
    xp_d = nc.dram_tensor("xpin", [Bc * 63], f32, kind="ExternalInput").ap()
    cw_d = nc.dram_tensor("cw", [128, Fw], bf16, kind="ExternalInput").ap()
    cb_d = nc.dram_tensor("cb", [128, 8], f32, kind="ExternalInput").ap()
    y_d = nc.dram_tensor("yout", [2, P, G], f32, kind="ExternalOutput").ap()

    xv = x_d.rearrange("(p f) -> p f", p=P)     # [128, G*9]
    xpv = xp_d.rearrange("(p f) -> p f", p=P)   # [128, G*63]
    yv = y_d                                     # [2, 128, 128]

    NCH = 4                  # dma chunks over g
    GC = G // NCH            # 32 g per chunk
    NBLK = 4                 # blocks per chunk (8 g each)
    GB = GC // NBLK          # 8 g per block
    NGRP = 2                 # groups per block (4 g each)
    GG = GB // NGRP          # 4

    wqp_l = [float(v) for v in wqp[0]]
    wkp_l = [float(v) for v in wkp[0]]

    with tile.TileContext(nc) as tc:
        with (
            tc.tile_pool(name="consts", bufs=1) as pc,
            tc.tile_pool(name="pin", bufs=2) as pin,
            tc.tile_pool(name="pq", bufs=2) as pq,
            tc.tile_pool(name="patt", bufs=2) as pa,
            tc.tile_pool(name="pmm", bufs=2) as pm,
            tc.tile_pool(name="py2", bufs=6) as py2,
            tc.tile_pool(name="pys", bufs=2) as pys,
            tc.tile_pool(name="pps", bufs=2, space="PSUM") as pps,
        ):
            cw_t = pc.tile([128, Fw], bf16)
            nc.sync.dma_start(cw_t[:], cw_d)
            cb_t = pc.tile([128, 8], f32)
            nc.sync.dma_start(cb_t[:], cb_d)

            def wslice(name):
                o, w_, r1, r0 = cols[name]
                return cw_t[r0:r1, o:o + w_]

            ident = wslice('id')

            for k in range(NCH):
                # ---- input DMA for this chunk ----
                xpc = pin.tile([128, GC * 63 + 16], bf16, tag="xp")
                nc.gpsimd.memset(xpc[:, GC * 63:], 0.0)
                nc.gpsimd.dma_start(xpc[:, 0:GC * 63], xpv[:, k * GC * 63:(k + 1) * GC * 63])
                xc = pin.tile([128, GC * 9], bf16, tag="x")
                nc.gpsimd.dma_start(xc[:], xv[:, k * GC * 9:(k + 1) * GC * 9])

                # ---- qp / kp for the whole chunk (32 g) ----
                Q = pq.tile([128, GC * 18], bf16)
                qview = Q.rearrange("p (g t) -> p g t", t=18)
                xp3 = xpc[:, 0:GC * 63].rearrange("p (g c n) -> p g c n", c=7, n=9)
                for c in range(7):
                    if c == 0:
                        nc.vector.tensor_scalar_mul(
                            qview[:, :, 0:9], xp3[:, :, c, :], wqp_l[c])
                        nc.vector.tensor_scalar_mul(
                            qview[:, :, 9:18], xp3[:, :, c, :], wkp_l[c])
                    else:
                        nc.vector.scalar_tensor_tensor(
                            qview[:, :, 0:9], xp3[:, :, c, :], wqp_l[c],
                            qview[:, :, 0:9], op0=ALU.mult, op1=ALU.add)
                        nc.vector.scalar_tensor_tensor(
                            qview[:, :, 9:18], xp3[:, :, c, :], wkp_l[c],
                            qview[:, :, 9:18], op0=ALU.mult, op1=ALU.add)

                for j in range(NBLK):
                    gb0 = j * GB  # g offset within chunk
                    # views for this block (8 g)
                    xpb = xpc[:, gb0 * 63:(gb0 + GB) * 63].rearrange(
                        "p (g c n) -> p g c n", c=7, n=9)
                    xb = xc[:, gb0 * 9:(gb0 + GB) * 9].rearrange(
                        "p (g n) -> p g n", n=9)
                    qb = Q[:, gb0 * 18:(gb0 + GB) * 18].rearrange(
                        "p (g t) -> p g t", t=18)

                    # --- param branch energy + softmax pieces (n padded to 10) ---
                    E = pa.tile([128, GB * 90], bf16, tag="E")
                    E4 = E.rearrange("p (g i n) -> p g i n", i=9, n=10)
                    nc.gpsimd.memset(E4[:, :, :, 9], 0.0)
                    qpA = qb[:, :, 0:9].unsqueeze(3).broadcast_to((128, GB, 9, 9))
                    kpA = qb[:, :, 9:18].unsqueeze(2).broadcast_to((128, GB, 9, 9))
                    nc.gpsimd.tensor_mul(E4[:, :, :, 0:9], qpA, kpA)
                    E2 = pa.tile([128, GB * 90], bf16, tag="E2")
                    E24 = E2.rearrange("p (g i n) -> p g i n", i=9, n=10)
                    nc.scalar.activation(E2[:], E[:], AF.Exp)

                    D = pa.tile([128, GB * 18], f32, tag="D")
                    Dv = D.rearrange("p (g t) -> p g t", t=18)
                    nc.vector.tensor_reduce(
                        Dv[:, :, 0:9], E24[:, :, :, 0:9], axis=mybir.AxisListType.X, op=ALU.add)

                    # --- image branch energy ---
                    EI = pa.tile([128, GB * 81], bf16, tag="EI")
                    EI4 = EI.rearrange("p (g i n) -> p g i n", i=9, n=9)
                    xiA = xb.unsqueeze(3).broadcast_to((128, GB, 9, 9))
                    xnA = xb.unsqueeze(2).broadcast_to((128, GB, 9, 9))
                    nc.gpsimd.tensor_mul(EI4, xiA, xnA)
                    EI2 = pa.tile([128, GB * 81], bf16, tag="EI2")
                    EI24 = EI2.rearrange("p (g i n) -> p g i n", i=9, n=9)
                    nc.scalar.activation(EI2[:], EI[:], AF.Exp, scale=c0)
                    nc.vector.tensor_reduce(
                        Dv[:, :, 9:18], EI24, axis=mybir.AxisListType.X, op=ALU.add)

                    R = pa.tile([128, GB * 18], f32, tag="R")
                    nc.vector.reciprocal(R[:], D[:])
                    Rv = R.rearrange("p (g t) -> p g t", t=18)

                    # --- bilinears (T2 padded to n=10 for the 2x DVE mode) ---
                    T2 = pa.tile([128, GB * 630], bf16, tag="T2")
                    T25 = T2.rearrange("p (g c i n) -> p g c i n", c=7, i=9, n=10)
                    xpb_u = xpb.unsqueeze(3)
                    ap10 = [list(p) for p in xpb_u.ap]
                    ap10[-1][1] = 10  # read 10 consecutive (1 slack elem, unused)
                    xpbA = bass.AP(xpb_u.tensor, xpb_u.offset, ap10).broadcast_to(
                        (128, GB, 7, 9, 10))
                    e2A = E24.unsqueeze(2).broadcast_to((128, GB, 7, 9, 10))
                    nc.vector.tensor_mul(T25, xpbA, e2A)
                    GT = pa.tile([128, GB * 63], f32, tag="GT")
                    GT4 = GT.rearrange("p (g c i) -> p g c i", c=7, i=9)
                    nc.vector.tensor_reduce(
                        GT4, T25[:, :, :, :, 0:9], axis=mybir.AxisListType.X, op=ALU.add)

                    TI = pa.tile([128, GB * 81], bf16, tag="TI")
                    TI4 = TI.rearrange("p (g i n) -> p g i n", i=9, n=9)
                    xnA2 = xb.unsqueeze(2).broadcast_to((128, GB, 9, 9))
                    nc.gpsimd.tensor_mul(TI4, xnA2, EI24)
                    GI = pa.tile([128, GB * 9], f32, tag="GI")
                    GI3 = GI.rearrange("p (g i) -> p g i", i=9)
                    nc.vector.tensor_reduce(
                        GI3, TI4, axis=mybir.AxisListType.X, op=ALU.add)

                    # --- attention outputs, laid out (g, i*8+c) for transpose ---
                    OCt = pa.tile([128, GB * 72], bf16, tag="OC")
                    OCv = OCt.rearrange("p (g i c) -> p g i c", i=9, c=8)
                    # param: out[g, i, c<7] = GT[g,c,i] * R[g,i]
                    ocp = OCv[:, :, :, 0:7].transpose([0, 1, 3, 2])  # p g c i
                    rpA = Rv[:, :, 0:9].unsqueeze(2).broadcast_to((128, GB, 7, 9))
                    nc.gpsimd.tensor_mul(ocp, GT4, rpA)
                    # image: out[g, i, 7] = GI[g,i] * R[g, 9+i]
                    nc.gpsimd.tensor_mul(OCv[:, :, :, 7], GI3, Rv[:, :, 9:18])

                    # ---- PE stage: 2 groups of 4 g ----
                    for h2 in range(NGRP):
                        ggl = [h2 * GG + t for t in range(GG)]
                        ps_tr = pps.tile([128, 512], bf16, tag="trans")
                        for t, gg in enumerate(ggl):
                            nc.tensor.transpose(
                                ps_tr[0:72, t * 128:(t + 1) * 128],
                                OCt[:, gg * 72:(gg + 1) * 72],
                                ident)
                        oct = pm.tile([128, 512], bf16, tag="oct")
                        nc.vector.tensor_copy(oct[0:72, :], ps_tr[0:72, :])
                        octr = oct

                        y2l = []
                        for t in range(5):
                            ps1 = pps.tile([128, 512], f32, tag="y1")
                            if t < 4:
                                nc.tensor.matmul(
                                    ps1[:], wslice(f'w1quad{t}'),
                                    octr[0:64, :],
                                    start=True, stop=True)
                                y1sb = pm.tile([128, 512], bf16, tag="y1sb")
                                nc.scalar.activation(
                                    y1sb[:], ps1[:], AF.Relu, bias=cb_t[:, 0:1])
                                ps2 = pps.tile([128, 512], f32, tag="y2")
                                nc.tensor.matmul(
                                    ps2[:], wslice('w2pair'),
                                    y1sb[:],
                                    start=True, stop=True)
                                y2sb = py2.tile([128, 512], bf16, tag="y2sb")
                                if t < 3:
                                    nc.vector.tensor_scalar(
                                        y2sb[:], ps2[:], cb_t[:, 1:2], 0.0,
                                        op0=ALU.add, op1=ALU.max)
                                else:
                                    nc.scalar.activation(
                                        y2sb[:], ps2[:], AF.Relu, bias=cb_t[:, 1:2])
                            else:
                                nc.tensor.matmul(
                                    ps1[0:64, :], wslice('w1s'),
                                    octr[64:72, :], start=True, stop=True)
                                y1sb = pm.tile([128, 512], bf16, tag="y1sb")
                                nc.scalar.activation(
                                    y1sb[0:64, :], ps1[0:64, :], AF.Relu,
                                    bias=cb_t[0:64, 5:6])
                                ps2 = pps.tile([128, 512], f32, tag="y2")
                                nc.tensor.matmul(
                                    ps2[0:64, :], wslice('w2s'),
                                    y1sb[0:64, :],
                                    start=True, stop=True)
                                y2sb = py2.tile([128, 512], bf16, tag="y2sb")
                                nc.scalar.activation(
                                    y2sb[0:64, :], ps2[0:64, :], AF.Relu,
                                    bias=cb_t[0:64, 6:7])
                            y2l.append(y2sb)

                        hs = []
                        for ch in range(2):
                            sfx = 'a' if ch == 0 else 'b'
                            psh = pps.tile([128, 512], f32, tag="h")
                            for t in range(5):
                                kk = 128 if t < 4 else 64
                                nc.tensor.matmul(
                                    psh[:], wslice(f'm1_{t}{sfx}'),
                                    y2l[t][0:kk, :],
                                    start=(t == 0), stop=(t == 4))
                            hsb = pm.tile([128, 512], bf16, tag="hsb")
                            nc.scalar.activation(
                                hsb[:], psh[:], AF.Relu, bias=cb_t[:, 2 + ch:3 + ch])
                            hs.append(hsb)

                        psy = pps.tile([2, 512], f32, tag="y2")
                        nc.tensor.matmul(psy[:], wslice('fw2a'),
                                         hs[0][:], start=True, stop=False)
                        nc.tensor.matmul(psy[:], wslice('fw2b'),
                                         hs[1][:], start=False, stop=True)

                        # sigmoid(x) = 0.5*tanh(0.5*x) + 0.5  (keeps exp table set)
                        if j == 0 and h2 == 0:
                            Ysb = pys.tile([2, GC * 128], f32, tag="Y")
                        gl0 = j * GB + h2 * GG  # g offset within chunk
                        yview = Ysb.rearrange("c (b g) -> c g b", g=GC)
                        src = psy.rearrange("c (g b) -> c g b", g=GG)
                        nc.scalar.activation(
                            yview[:, gl0:gl0 + GG, :], src, AF.Tanh, scale=0.5,
                            bias=cb_t[0:2, 4:5])

                # ---- output DMA for the chunk ----
                ydst = yv[:, :, k * GC:(k + 1) * GC]  # [2, 128, GC]
                ysrc = Ysb.rearrange("c (b g) -> c b g", g=GC)
                nc.sync.dma_start(ydst, ysrc)

    nc.compile()

    in_maps = []
    for core in range(NC):
        sl = slice(core * Bc, (core + 1) * Bc)
        in_maps.append({
            "xin": np.ascontiguousarray(x[sl].reshape(-1)),
            "xpin": np.ascontiguousarray(xp[sl].reshape(-1)),
            "cw": cw_np_bf,
            "cb": cb_np,
        })
    return nc, in_maps


def kernel(**inputs):
    from concourse.bass_utils import run_bass_kernel_spmd
    nc, in_maps = _build(inputs)
    kernel._last_nc = nc
    res = run_bass_kernel_spmd(nc, in_maps, core_ids=list(range(NC)))
    kernel._last_result = res
    Bc = in_maps[0]["xin"].size // 9
    outs = []
    for core in range(NC):
        yc = res.results[core]["yout"]          # [2, 128, G] = (cls, p, g)
        outs.append(yc.transpose(1, 2, 0).reshape(-1) * 0.5 + 0.5)  # rows r=p*G+g
    return np.concatenate(outs)

